# revision 11
# baseline (speedup 1.0000x reference)
"""Trainium2 Bass kernel for nn_ADAM_SINDy_MODEL (568-term SINDy library regression).

Math: the reference computes terms[B,T,568] @ a with a data-independent
column mask. Since the mask and all library indices depend only on
(a, uses_self, *_idx), the whole thing collapses per row to

    out = c0*con + w_lin.x + w_hill.g + x^T (U x + W_drug d + W_mm2^T g)

with g = x/(0.5+x) = 1 - r, r = 1/(2x+1).  Substituting g = 1-r and
folding constants, with the feature vector u = [con, x(21), d(5), r(21), 1]:

    H = W1^T u          (22 values per row; H_0 = 0)
    out = w71 . [u ; (u[0:22] * H)]

which is two small matmuls + one elementwise multiply per row tile.

Per core (data parallel over batch): 32768 rows, processed in 64 chunks of
512 rows.  Each chunk: DMA in -> compute r -> 4x TensorE transpose (128x49
-> 49x128) -> evac to SBUF -> mm1 (W1) -> P-mult -> 4x reduce-matmul
(data-as-weights, output lands [128,4] across partitions) -> copy out -> DMA.

Row mapping within a chunk: local row = chunk*512 + 4*p + q  (p=partition,
q=sub-block) so the final [128,4] output tile DMAs out contiguously.
"""

import os
import sys

import numpy as np

if "/opt/trn_rl_repo" not in sys.path:
    sys.path.insert(0, "/opt/trn_rl_repo")

NX, ND = 21, 5
B, T = 128, 2048
NCORES = 8
BPC = B // NCORES          # batches per core
ROWS = BPC * T             # rows per core
CHUNK = 512
NCHUNK = ROWS // CHUNK
NBLK = 4                   # 128-row sub-blocks per chunk
FEAT = 49                  # con, x(21), d(5), r(21), ones
NH = 22                    # H rows (dummy + 21)
PBASE = 64                 # partition where P is stacked (must be 32-aligned)
TALL = PBASE + NH          # u^T stacked with P

_CACHE = {}


def _build_coeffs(a, lin_idx, drug_idx, bilin_idx, mm2_idx, hill_idx, uses_self):
    a = np.asarray(a, np.float64).reshape(-1)
    uses_self = np.asarray(uses_self).astype(bool).reshape(-1)
    lin_idx = np.asarray(lin_idx).reshape(-1)
    drug_idx = np.asarray(drug_idx).reshape(-1, 2)
    bilin_idx = np.asarray(bilin_idx).reshape(-1, 2)
    mm2_idx = np.asarray(mm2_idx).reshape(-1, 2)
    hill_idx = np.asarray(hill_idx).reshape(-1)

    n = a.shape[0]
    idx = np.arange(n)
    zero = np.where(uses_self, a > 0.0, a < 0.0) & (idx >= 2)
    ae = np.where(zero, 0.0, a)

    nl = len(lin_idx)
    ndg = len(drug_idx)
    nb = len(bilin_idx)
    nm = len(mm2_idx)
    nh = len(hill_idx)
    o1 = 1
    o2 = o1 + nl
    o3 = o2 + ndg
    o4 = o3 + nb
    o5 = o4 + nm

    c0 = ae[0]
    w_lin = np.zeros(NX)
    np.add.at(w_lin, lin_idx, ae[o1:o2])
    W_drug = np.zeros((NX, ND))
    np.add.at(W_drug, (drug_idx[:, 0], drug_idx[:, 1]), ae[o2:o3])
    U = np.zeros((NX, NX))
    np.add.at(U, (bilin_idx[:, 0], bilin_idx[:, 1]), ae[o3:o4])
    W_mm2 = np.zeros((NX, NX))
    np.add.at(W_mm2, (mm2_idx[:, 0], mm2_idx[:, 1]), ae[o4:o5])
    w_hill = np.zeros(NX)
    np.add.at(w_hill, hill_idx, ae[o5 : o5 + nh])

    # u layout: p0=con, p1..21=x, p22..26=d, p27=1, p28..48=r
    W1 = np.zeros((FEAT, NH))
    W1[1 : 1 + NX, 1 : 1 + NX] = U.T
    W1[22 : 22 + ND, 1 : 1 + NX] = W_drug.T
    W1[28 : 28 + NX, 1 : 1 + NX] = -W_mm2

    v49 = np.zeros(FEAT)
    v49[0] = c0
    v49[1 : 1 + NX] = w_lin + W_mm2.sum(axis=0)
    v49[27] = w_hill.sum()
    v49[28 : 28 + NX] = -w_hill

    w71 = np.concatenate([v49, np.zeros(PBASE - FEAT), np.ones(NH)])
    return W1.astype(np.float32), w71.astype(np.float32).reshape(TALL, 1)


def _build_nc():
    import concourse.bacc as bacc
    import concourse.bass as bass
    import concourse.tile as tile
    from concourse import mybir

    f32 = mybir.dt.float32
    f32r = mybir.dt.float32r
    Act = mybir.ActivationFunctionType
    Alu = mybir.AluOpType

    nc = bacc.Bacc(
        "TRN2", target_bir_lowering=False, debug=False, num_devices=NCORES
    )
    cand = nc.declare_dram_parameter("cand", [ROWS, 28], f32, isOutput=False)
    w1_d = nc.declare_dram_parameter("w1", [FEAT, NH], f32r, isOutput=False)
    w71_d = nc.declare_dram_parameter("w71", [TALL, 1], f32, isOutput=False)
    iden_d = nc.declare_dram_parameter("iden", [128, 128], f32, isOutput=False)
    out_d = nc.declare_dram_parameter("out", [ROWS], f32, isOutput=True)

    with tile.TileContext(nc) as tc:
        with (
            tc.tile_pool(name="const", bufs=1) as cpool,
            tc.tile_pool(name="rowbuf", bufs=3) as rpool,
            tc.tile_pool(name="tall", bufs=3) as tpool,
            tc.tile_pool(name="osb", bufs=4) as opool,
            tc.tile_pool(name="psA", bufs=2, space="PSUM") as psA,
            tc.tile_pool(name="psH", bufs=2, space="PSUM") as psH,
            tc.tile_pool(name="psO", bufs=2, space="PSUM") as psO,
        ):
            w1_sb = cpool.tile([FEAT, NH], f32r)
            nc.sync.dma_start(out=w1_sb[:], in_=w1_d[:, :])
            w71_sb = cpool.tile([TALL, 1], f32)
            nc.sync.dma_start(out=w71_sb[:], in_=w71_d[:, :])
            iden_sb = cpool.tile([128, 128], f32)
            nc.sync.dma_start(out=iden_sb[:], in_=iden_d[:, :])

            for c in range(NCHUNK):
                r0 = c * CHUNK
                rowbuf = rpool.tile([128, NBLK * FEAT], f32)
                rb3 = rowbuf[:].rearrange("p (q f) -> p q f", q=NBLK)
                # DRAM row r0 + 4p + q -> partition p, block q
                src = cand[r0 : r0 + CHUNK, :].rearrange("(p q) f -> p q f", q=NBLK)
                nc.sync.dma_start(out=rb3[:, :, 0:28], in_=src)
                # t = 2x+1 -> r = 1/t
                nc.vector.tensor_scalar(
                    out=rb3[:, :, 28:49],
                    in0=rb3[:, :, 1:22],
                    scalar1=2.0,
                    scalar2=1.0,
                    op0=Alu.mult,
                    op1=Alu.add,
                )
                nc.vector.reciprocal(
                    out=rb3[:, :, 28:49], in_=rb3[:, :, 28:49]
                )

                uT_ps = psA.tile([FEAT, CHUNK], f32)
                for q in range(NBLK):
                    nc.tensor.transpose(
                        out=uT_ps[:, q * 128 : (q + 1) * 128],
                        in_=rb3[:, q, :],
                        identity=iden_sb[:],
                    )

                tall = tpool.tile([TALL, CHUNK], f32r)
                nc.scalar.activation(
                    out=tall[0:FEAT, :], in_=uT_ps[:], func=Act.Copy
                )

                H = psH.tile([NH, CHUNK], f32)
                nc.tensor.matmul(
                    out=H[:],
                    lhsT=w1_sb[:],
                    rhs=tall[0:FEAT, :],
                    start=True,
                    stop=True,
                )

                # P = u[0:22] * H  (stacked under u^T)
                nc.vector.tensor_mul(
                    out=tall[PBASE : PBASE + NH, :],
                    in0=tall[0:NH, :],
                    in1=H[:],
                )

                o128 = psO.tile([128, NBLK], f32)
                for q in range(NBLK):
                    nc.tensor.matmul(
                        out=o128[:, q : q + 1],
                        lhsT=tall[:, q * 128 : (q + 1) * 128].bitcast(f32),
                        rhs=w71_sb[:],
                        start=True,
                        stop=True,
                        skip_group_check=True,
                    )

                osb = opool.tile([128, NBLK], f32)
                nc.scalar.activation(out=osb[:], in_=o128[:], func=Act.Copy)
                dst = out_d[r0 : r0 + CHUNK].rearrange("(p q) -> p q", q=NBLK)
                nc.sync.dma_start(out=dst, in_=osb[:])

    nc.compile()
    return nc


def _get_nc():
    if "nc" not in _CACHE:
        _CACHE["nc"] = _build_nc()
    return _CACHE["nc"]


def _ensure_ntff_hook():
    """The agent image's antenv lacks axon_hooks; synthesize it from the
    boot module's ctypes NTFF driver so trace=True can capture profiles."""
    try:
        from antenv.axon_hooks import get_axon_ntff_profile_hook  # noqa: F401

        return
    except ImportError:
        pass
    try:
        import types

        import antenv
        from trn_agent_boot.trn_boot import _ntff_profile_via_ctypes

        hook = _ntff_profile_via_ctypes("/opt/axon/libaxon_pjrt.so")
        mod = types.ModuleType("antenv.axon_hooks")
        holder = {"hook": hook}
        mod.get_axon_ntff_profile_hook = lambda: holder["hook"]
        mod.set_axon_ntff_profile_hook = lambda h: holder.update(hook=h)
        sys.modules["antenv.axon_hooks"] = mod
        antenv.axon_hooks = mod
    except Exception as e:  # degrade to untraced
        print(f"ntff hook setup failed: {e}", file=sys.stderr)


def kernel(**inputs) -> np.ndarray:
    from concourse.bass_utils import run_bass_kernel_spmd

    cand = np.ascontiguousarray(np.asarray(inputs["candidates"], dtype=np.float32))
    assert cand.shape == (B, T, 27), cand.shape
    W1, w71 = _build_coeffs(
        inputs["a"],
        inputs["lin_idx"],
        inputs["drug_idx"],
        inputs["bilin_idx"],
        inputs["mm2_idx"],
        inputs["hill_idx"],
        inputs["uses_self"],
    )
    iden = np.eye(128, dtype=np.float32)

    nc = _get_nc()
    in_maps = []
    for i in range(NCORES):
        shard = cand[i * BPC : (i + 1) * BPC].reshape(ROWS, 27)
        shard = np.ascontiguousarray(
            np.concatenate([shard, np.ones((ROWS, 1), np.float32)], axis=1)
        )
        in_maps.append({"cand": shard, "w1": W1, "w71": w71, "iden": iden})

    trace = os.environ.get("BASS_TRACE", "") == "1"
    if trace:
        _ensure_ntff_hook()
    res = run_bass_kernel_spmd(
        nc, in_maps, core_ids=list(range(NCORES)), trace=trace
    )
    if res.exec_time_ns is not None:
        print(f"HW exec time: {res.exec_time_ns} ns")
        _CACHE["exec_time_ns"] = res.exec_time_ns

    out = np.concatenate(
        [res.results[i]["out"].reshape(BPC, T) for i in range(NCORES)], axis=0
    )
    return out.astype(np.float32)


# revision 14
# speedup vs baseline: 1.4777x; 1.4777x over previous
"""Trainium2 Bass kernel for nn_ADAM_SINDy_MODEL (568-term SINDy library regression).

Math: the reference computes terms[B,T,568] @ a with a data-independent
column mask. Since the mask and all library indices depend only on
(a, uses_self, *_idx), the whole thing collapses per row to

    out = c0*con + w_lin.x + w_hill.g + x^T (U x + W_drug d + W_mm2^T g)

with g = x/(0.5+x) = 1 - r, r = 1/(2x+1).  Substituting g = 1-r and
folding constants, with the feature vector u = [con, x(21), d(5), r(21), 1]:

    H = W1^T u          (22 values per row; H_0 = 0)
    out = w71 . [u ; (u[0:22] * H)]

which is two small matmuls + one elementwise multiply per row tile.

Per core (data parallel over batch): 32768 rows, processed in 64 chunks of
512 rows.  Each chunk: DMA in -> compute r -> 4x TensorE transpose (128x49
-> 49x128) -> evac to SBUF -> mm1 (W1) -> P-mult -> 4x reduce-matmul
(data-as-weights, output lands [128,4] across partitions) -> copy out -> DMA.

Row mapping within a chunk: local row = chunk*512 + 4*p + q  (p=partition,
q=sub-block) so the final [128,4] output tile DMAs out contiguously.
"""

import os
import sys

import numpy as np

if "/opt/trn_rl_repo" not in sys.path:
    sys.path.insert(0, "/opt/trn_rl_repo")

NX, ND = 21, 5
B, T = 128, 2048
NCORES = 8
BPC = B // NCORES          # batches per core
ROWS = BPC * T             # rows per core
CHUNK = 512
NCHUNK = ROWS // CHUNK
NBLK = 4                   # 128-row sub-blocks per chunk
FEAT = 49                  # con, x(21), d(5), r(21), ones
NH = 22                    # H rows (dummy + 21)
PBASE = 64                 # partition where P is stacked (must be 32-aligned)
TALL = PBASE + NH          # u^T stacked with P

_CACHE = {}


def _build_coeffs(a, lin_idx, drug_idx, bilin_idx, mm2_idx, hill_idx, uses_self):
    a = np.asarray(a, np.float64).reshape(-1)
    uses_self = np.asarray(uses_self).astype(bool).reshape(-1)
    lin_idx = np.asarray(lin_idx).reshape(-1)
    drug_idx = np.asarray(drug_idx).reshape(-1, 2)
    bilin_idx = np.asarray(bilin_idx).reshape(-1, 2)
    mm2_idx = np.asarray(mm2_idx).reshape(-1, 2)
    hill_idx = np.asarray(hill_idx).reshape(-1)

    n = a.shape[0]
    idx = np.arange(n)
    zero = np.where(uses_self, a > 0.0, a < 0.0) & (idx >= 2)
    ae = np.where(zero, 0.0, a)

    nl = len(lin_idx)
    ndg = len(drug_idx)
    nb = len(bilin_idx)
    nm = len(mm2_idx)
    nh = len(hill_idx)
    o1 = 1
    o2 = o1 + nl
    o3 = o2 + ndg
    o4 = o3 + nb
    o5 = o4 + nm

    c0 = ae[0]
    w_lin = np.zeros(NX)
    np.add.at(w_lin, lin_idx, ae[o1:o2])
    W_drug = np.zeros((NX, ND))
    np.add.at(W_drug, (drug_idx[:, 0], drug_idx[:, 1]), ae[o2:o3])
    U = np.zeros((NX, NX))
    np.add.at(U, (bilin_idx[:, 0], bilin_idx[:, 1]), ae[o3:o4])
    W_mm2 = np.zeros((NX, NX))
    np.add.at(W_mm2, (mm2_idx[:, 0], mm2_idx[:, 1]), ae[o4:o5])
    w_hill = np.zeros(NX)
    np.add.at(w_hill, hill_idx, ae[o5 : o5 + nh])

    # u layout: p0=con, p1..21=x, p22..26=d, p27=1, p28..48=r
    W1 = np.zeros((FEAT, NH))
    W1[1 : 1 + NX, 1 : 1 + NX] = U.T
    W1[22 : 22 + ND, 1 : 1 + NX] = W_drug.T
    W1[28 : 28 + NX, 1 : 1 + NX] = -W_mm2

    v49 = np.zeros(FEAT)
    v49[0] = c0
    v49[1 : 1 + NX] = w_lin + W_mm2.sum(axis=0)
    v49[27] = w_hill.sum()
    v49[28 : 28 + NX] = -w_hill

    w71 = np.concatenate([v49, np.zeros(PBASE - FEAT), np.ones(NH)])
    return W1.astype(np.float16), w71.astype(np.float16).reshape(TALL, 1)


def _build_nc():
    import concourse.bacc as bacc
    import concourse.tile as tile
    from concourse import mybir

    f32 = mybir.dt.float32
    f16 = mybir.dt.float16
    Act = mybir.ActivationFunctionType

    nc = bacc.Bacc(
        "TRN2", target_bir_lowering=False, debug=False, num_devices=NCORES
    )
    cand = nc.declare_dram_parameter("cand", [ROWS, 28], f32, isOutput=False)
    w1_d = nc.declare_dram_parameter("w1", [FEAT, NH], f16, isOutput=False)
    w71_d = nc.declare_dram_parameter("w71", [TALL, 1], f16, isOutput=False)
    iden_d = nc.declare_dram_parameter("iden", [128, 128], f16, isOutput=False)
    out_d = nc.declare_dram_parameter("out", [ROWS], f32, isOutput=True)

    def act_recip(out, in_, scale, bias):
        """activation(Reciprocal) — the bass wrapper refuses Reciprocal
        outright (generic accuracy concern); our domain is 2x+1 in [1,3)
        where the 1016-bucket LUT is plenty accurate, so emit the
        InstActivation directly."""
        eng = nc.scalar
        ins = [eng.lower_ap(in_)]
        for arg in (bias, scale, 0.0):
            ins.append(mybir.ImmediateValue(dtype=mybir.dt.float32, value=arg))
        return eng.add_instruction(
            mybir.InstActivation(
                name=nc.get_next_instruction_name(),
                func=Act.Reciprocal,
                ins=ins,
                outs=[eng.lower_ap(out)],
            )
        )

    BS = 50  # fp16 block stride (keeps 4B alignment)

    with tile.TileContext(nc) as tc:
        with (
            tc.tile_pool(name="const", bufs=1) as cpool,
            tc.tile_pool(name="rowbuf", bufs=3) as rpool,
            tc.tile_pool(name="rf16", bufs=3) as rfpool,
            tc.tile_pool(name="tall", bufs=3) as tpool,
            tc.tile_pool(name="osb", bufs=4) as opool,
            tc.tile_pool(name="psA", bufs=2, space="PSUM") as psA,
            tc.tile_pool(name="psH", bufs=2, space="PSUM") as psH,
            tc.tile_pool(name="psO", bufs=2, space="PSUM") as psO,
        ):
            w1_sb = cpool.tile([FEAT, NH], f16)
            nc.sync.dma_start(out=w1_sb[:], in_=w1_d[:, :])
            w71_sb = cpool.tile([TALL, 1], f16)
            nc.sync.dma_start(out=w71_sb[:], in_=w71_d[:, :])
            iden_sb = cpool.tile([128, 128], f16)
            nc.sync.dma_start(out=iden_sb[:], in_=iden_d[:, :])

            for c in range(NCHUNK):
                r0 = c * CHUNK
                rowbuf = rpool.tile([128, NBLK * 28], f32)
                rb3 = rowbuf[:].rearrange("p (q f) -> p q f", q=NBLK)
                # DRAM row r0 + 4p + q -> partition p, block q
                src = cand[r0 : r0 + CHUNK, :].rearrange("(p q) f -> p q f", q=NBLK)
                nc.sync.dma_start(out=rb3[:, :, :], in_=src)

                rf16 = rfpool.tile([128, NBLK * BS], f16)
                rf3 = rf16[:].rearrange("p (q f) -> p q f", q=NBLK)
                # cast con/x/d/1 to fp16
                nc.vector.tensor_copy(out=rf3[:, :, 0:28], in_=rb3[:, :, :])
                # r = 1/(2x+1)  (DVE fallback while isolating a HW fault)
                nc.vector.tensor_scalar(
                    out=rb3[:, :, 1:22],
                    in0=rb3[:, :, 1:22],
                    scalar1=2.0,
                    scalar2=1.0,
                    op0=mybir.AluOpType.mult,
                    op1=mybir.AluOpType.add,
                )
                with nc.allow_low_precision(reason="r in [1/3,1], fp16 ok"):
                    nc.vector.reciprocal(
                        out=rf3[:, :, 28:49], in_=rb3[:, :, 1:22]
                    )

                uT_ps = psA.tile([FEAT, CHUNK], f16)
                for q in range(NBLK):
                    nc.tensor.transpose(
                        out=uT_ps[:, q * 128 : (q + 1) * 128],
                        in_=rf3[:, q, 0:FEAT],
                        identity=iden_sb[:],
                    )

                tall = tpool.tile([TALL, CHUNK], f16)
                nc.scalar.activation(
                    out=tall[0:FEAT, :], in_=uT_ps[:], func=Act.Copy
                )

                H = psH.tile([NH, CHUNK], f32)
                nc.tensor.matmul(
                    out=H[:],
                    lhsT=w1_sb[:],
                    rhs=tall[0:FEAT, :],
                    start=True,
                    stop=True,
                )

                # P = u[0:22] * H  (stacked under u^T at PBASE)
                nc.vector.tensor_mul(
                    out=tall[PBASE : PBASE + NH, :],
                    in0=tall[0:NH, :],
                    in1=H[:],
                )

                o128 = psO.tile([128, NBLK], f32)
                for q in range(NBLK):
                    nc.tensor.matmul(
                        out=o128[:, q : q + 1],
                        lhsT=tall[:, q * 128 : (q + 1) * 128],
                        rhs=w71_sb[:],
                        start=True,
                        stop=True,
                        skip_group_check=True,
                    )

                osb = opool.tile([128, NBLK], f32)
                nc.scalar.activation(out=osb[:], in_=o128[:], func=Act.Copy)
                dst = out_d[r0 : r0 + CHUNK].rearrange("(p q) -> p q", q=NBLK)
                nc.sync.dma_start(out=dst, in_=osb[:])

    nc.compile()
    return nc


def _get_nc():
    if "nc" not in _CACHE:
        _CACHE["nc"] = _build_nc()
    return _CACHE["nc"]


def _ensure_ntff_hook():
    """The agent image's antenv lacks axon_hooks; synthesize it from the
    boot module's ctypes NTFF driver so trace=True can capture profiles."""
    try:
        from antenv.axon_hooks import get_axon_ntff_profile_hook  # noqa: F401

        return
    except ImportError:
        pass
    try:
        import types

        import antenv
        from trn_agent_boot.trn_boot import _ntff_profile_via_ctypes

        hook = _ntff_profile_via_ctypes("/opt/axon/libaxon_pjrt.so")
        mod = types.ModuleType("antenv.axon_hooks")
        holder = {"hook": hook}
        mod.get_axon_ntff_profile_hook = lambda: holder["hook"]
        mod.set_axon_ntff_profile_hook = lambda h: holder.update(hook=h)
        sys.modules["antenv.axon_hooks"] = mod
        antenv.axon_hooks = mod
    except Exception as e:  # degrade to untraced
        print(f"ntff hook setup failed: {e}", file=sys.stderr)


def kernel(**inputs) -> np.ndarray:
    from concourse.bass_utils import run_bass_kernel_spmd

    cand = np.ascontiguousarray(np.asarray(inputs["candidates"], dtype=np.float32))
    assert cand.shape == (B, T, 27), cand.shape
    W1, w71 = _build_coeffs(
        inputs["a"],
        inputs["lin_idx"],
        inputs["drug_idx"],
        inputs["bilin_idx"],
        inputs["mm2_idx"],
        inputs["hill_idx"],
        inputs["uses_self"],
    )
    iden = np.eye(128, dtype=np.float16)

    nc = _get_nc()
    in_maps = []
    for i in range(NCORES):
        shard = cand[i * BPC : (i + 1) * BPC].reshape(ROWS, 27)
        shard = np.ascontiguousarray(
            np.concatenate([shard, np.ones((ROWS, 1), np.float32)], axis=1)
        )
        in_maps.append({"cand": shard, "w1": W1, "w71": w71, "iden": iden})

    trace = os.environ.get("BASS_TRACE", "") == "1"
    if trace:
        _ensure_ntff_hook()
    res = run_bass_kernel_spmd(
        nc, in_maps, core_ids=list(range(NCORES)), trace=trace
    )
    if res.exec_time_ns is not None:
        print(f"HW exec time: {res.exec_time_ns} ns")
        _CACHE["exec_time_ns"] = res.exec_time_ns

    out = np.concatenate(
        [res.results[i]["out"].reshape(BPC, T) for i in range(NCORES)], axis=0
    )
    return out.astype(np.float32)
